# revision 9
# baseline (speedup 1.0000x reference)
"""Trainium2 Bass kernel for an edge-weighted two-layer sparse MLP (QBAF).

Math (identical to the gather/segment_sum reference):
    out = sigmoid(x @ W1 + b1) @ W2 + b2
with W1 [2048, 1024] / W2 [1024, 8] densified on host from the sparse
edge lists (scatter-add; duplicate edges accumulate like segment_sum).

Sharding: data-parallel over batch — 8 cores x 512 rows each; weights
replicated.

Precision/layout: the 2048-deep layer-1 contraction is split into 8
pairs of 256 rows. The first FP8_PAIRS pairs run as fp8(e4m3) DoubleRow
matmuls (256 contraction rows per 512-cycle instruction -- 2x the
fp16 PE rate); the rest run fp16. Both operand sets are pre-scaled
(x*16, W1*256) so everything shares one fp32 PSUM accumulation per
neuron tile; the sigmoid activation descales by 2^-12. Measured
end-to-end rel err ~1.6e-2 vs the fp32 reference (tolerance 2e-2).

Schedule: m-outer over the 8 neuron tiles, with each m's fp16 pairs
trailing one m behind its fp8 run (fp16 operands arrive later) and the
layer-2 matmul of m trailing two runs behind. PSUM banks therefore
complete staggered ~2.2us apart, and the 8 ACT-engine sigmoids (~0.7us
each) hide under remaining layer-1 work instead of serializing at the
tail. Inputs stream over both HWDGE rings (SP + ACT) in PE-consumption
order; the ACT ring interleaves sigmoids between its DMA issues. PE
warmup matmuls cover the pre-data window so the 1.2->2.4GHz clock ramp
(~3us of sustained PE activity) completes just as real data lands.
"""

import sys

import numpy as np

if "/opt/trn_rl_repo" not in sys.path:
    sys.path.insert(0, "/opt/trn_rl_repo")

B = 4096
F = 2048
N1 = 1024
NT = 8
NCORES = 8
BSH = B // NCORES  # 512 batch rows per core
P = 128
M1 = N1 // P  # 8 neuron tiles
PAIRS = F // (2 * P)  # 8 contraction pairs of 256 rows
FP8_PAIRS = 6
FP16_PAIRS = PAIRS - FP8_PAIRS
SX = 16.0  # x pre-scale (power of 2; keeps fp8 out of subnormals)
SW = 256.0  # W1 pre-scale
WARMUP = 36  # narrow PE matmuls: the 1.2->2.4GHz HAM clock ramp needs
# ~3.5-4.5us of CONTINUOUS PE activity (gaps reset it), so warm up until
# the ramp is done AND the first m-blocks have landed, then never stall.
STRIP_END = False  # dropping the end-block EVSEM barrier deadlocks the
# end-block drains (they wait on events those EVSEMs set) -- keep it.

# DRAM m-block order: ring A streams m 0,2,4,6; ring B streams 1,3,5,7.
# Even-m blocks packed first so each ring's chunks are contiguous.
MORDER = [0, 2, 4, 6, 1, 3, 5, 7]
MPOS = {m: i for i, m in enumerate(MORDER)}

_CACHE = {}


def _build():
    import concourse.bass as bass
    import concourse.mybir as mybir
    import concourse.tile as tile

    dt = mybir.dt
    f8 = dt.float8e4
    f16 = dt.float16
    DR = mybir.MatmulPerfMode.DoubleRow
    SIG = mybir.ActivationFunctionType.Sigmoid

    nc = bass.Bass()
    x8 = nc.declare_dram_parameter("x8", [P, FP8_PAIRS * 2 * BSH], f8, isOutput=False)
    x16 = nc.declare_dram_parameter("x16", [P, FP16_PAIRS * 2 * BSH], f16, isOutput=False)
    w8 = nc.declare_dram_parameter("w8", [P, M1 * FP8_PAIRS * 2 * P], f8, isOutput=False)
    w16 = nc.declare_dram_parameter("w16", [P, M1 * FP16_PAIRS * 2 * P], f16, isOutput=False)
    w2c = nc.declare_dram_parameter("w2c", [P, M1 * NT], f16, isOutput=False)
    cn = nc.declare_dram_parameter("cn", [P, M1 + 1], dt.float32, isOutput=False)
    outT = nc.declare_dram_parameter("outT", [NT, BSH], dt.float32, isOutput=True)

    W8C = FP8_PAIRS * 2 * P  # w8 cols per m-block
    W16C = FP16_PAIRS * 2 * P

    with tile.TileContext(nc) as tc:
        with (
            tc.tile_pool(name="consts", bufs=1) as consts,
            tc.tile_pool(name="xp", bufs=1) as xp,
            tc.tile_pool(name="wp", bufs=1) as wp,
            tc.tile_pool(name="hp", bufs=M1) as hp,
            tc.tile_pool(name="outp", bufs=1) as outp,
            tc.tile_pool(name="ps", bufs=8, space="PSUM") as ps,
        ):
            x8t = {}  # pair j -> [P, 2, BSH] fp8 tile
            x16t = {}  # pair jj -> [P, 2, BSH] fp16 tile
            w8t = {}  # m -> list of (tile, tile_m_idx, j_lo, j_hi)
            w16t = {}  # m -> (tile, idx)

            def x8_dma(eng, j):
                t = xp.tile([P, 2, BSH], f8, tag=f"x8_{j}", name=f"x8_{j}")
                eng.dma_start(out=t[:], in_=x8[:, j * 2 * BSH : (j + 1) * 2 * BSH])
                x8t[j] = t

            def x16_dma(eng, jj):
                t = xp.tile([P, 2, BSH], f16, tag=f"x16_{jj}", name=f"x16_{jj}")
                eng.dma_start(out=t[:], in_=x16[:, jj * 2 * BSH : (jj + 1) * 2 * BSH])
                x16t[jj] = t

            def w8_dma(eng, m, jlo=0, jhi=FP8_PAIRS):
                np_ = jhi - jlo
                t = wp.tile([P, np_, 2, P], f8, tag=f"w8_{m}_{jlo}", name=f"w8_{m}_{jlo}")
                off = MPOS[m] * W8C + jlo * 2 * P
                eng.dma_start(out=t[:], in_=w8[:, off : off + np_ * 2 * P])
                w8t.setdefault(m, []).append((t, jlo, jhi))

            def w16_dma(eng, m):
                t = wp.tile([P, FP16_PAIRS, 2, P], f16, tag=f"w16_{m}", name=f"w16_{m}")
                off = MPOS[m] * W16C
                eng.dma_start(out=t[:], in_=w16[:, off : off + W16C])
                w16t[m] = t

            A = nc.sync
            Bq = nc.scalar

            # --- ring A (sync / SP): stream order = issue order
            w8_dma(A, 0, 0, 3)
            x8_dma(A, 1)
            x8_dma(A, 3)
            w8_dma(A, 0, 3, 6)
            x8_dma(A, 5)
            x8_dma(A, 4)
            w16_dma(A, 0)
            w8_dma(A, 2)
            w8_dma(A, 4)
            w8_dma(A, 6)
            w16_dma(A, 2)
            w16_dma(A, 4)
            w16_dma(A, 6)

            # --- ring B (scalar / ACT): DMA issues; the sigmoid chain and
            # the late weight blocks are interleaved further down.
            x8_dma(Bq, 0)
            x8_dma(Bq, 2)
            w8_dma(Bq, 1)
            x16_dma(Bq, 0)
            x16_dma(Bq, 1)
            cns = consts.tile([P, M1 + 1], dt.float32, tag="cn", name="cns")
            Bq.dma_start(out=cns[:], in_=cn[:])
            # dummy sigmoid: pulls the ACT table load off the critical path
            scr = consts.tile([P, 1], dt.float32, tag="scr", name="scr")
            bias0 = consts.tile([P, 1], dt.float32, tag="b0", name="bias0")
            nc.gpsimd.memset(bias0[:], 0.0)
            nc.scalar.activation(scr[:], bias0[:], SIG, bias=bias0[:], scale=1.0)
            w2s = consts.tile([P, M1 * NT], f16, tag="w2", name="w2s")
            Bq.dma_start(out=w2s[:], in_=w2c[:])
            w16_dma(Bq, 1)
            w8_dma(Bq, 3)

            hts = {}

            def sigmoid(m):
                ht = hp.tile([P, BSH], f16, tag="h", name=f"h{m}")
                nc.scalar.activation(
                    ht[:], accs[m][:], SIG, bias=cns[:, m : m + 1],
                    scale=1.0 / (SX * SW),
                )
                hts[m] = ht

            # --- PE program
            accs = [
                ps.tile([P, BSH], dt.float32, tag="acc", name=f"acc{m}")
                for m in range(M1)
            ]
            wsc = consts.tile([P, P], f16, tag="wsc", name="wsc")
            nc.gpsimd.memset(wsc[:], 0.0)
            for _ in range(WARMUP):
                nc.tensor.matmul(
                    accs[0][:, 0:P], wsc[:], wsc[:], start=True, stop=True
                )

            def l1_fp8(m):
                for t, jlo, jhi in w8t[m]:
                    for j in range(jlo, jhi):
                        nc.tensor.matmul(
                            accs[m][:],
                            t[:, j - jlo],
                            x8t[j][:],
                            start=(j == 0),
                            stop=False,
                            perf_mode=DR,
                        )

            def l1_fp16(m):
                t = w16t[m]
                for jj in range(FP16_PAIRS):
                    for s in range(2):
                        last = jj == FP16_PAIRS - 1 and s == 1
                        nc.tensor.matmul(
                            accs[m][:],
                            t[:, jj, s],
                            x16t[jj][:, s],
                            start=False,
                            stop=last,
                        )

            acc2 = ps.tile([P, BSH], dt.float32, tag="acc", name="acc2")

            def l2(m):
                nc.tensor.matmul(
                    acc2[:NT, :],
                    w2s[:, m * NT : (m + 1) * NT],
                    hts[m][:],
                    start=(m == 0),
                    stop=(m == M1 - 1),
                )

            # fp8 run of m; fp16 finish lags one m; L2 lags two. Sigmoids
            # (ACT) fire on each bank's stop; late B-ring DMA issues are
            # interleaved between them. Emission is chronological so Tile's
            # dependency tracking sees every accumulator write before its
            # sigmoid read.
            l1_fp8(0)
            l1_fp8(1)
            l1_fp16(0)  # stop m0
            sigmoid(0)
            w8_dma(Bq, 5)
            l1_fp8(2)
            l1_fp16(1)  # stop m1
            sigmoid(1)
            w16_dma(Bq, 3)
            l2(0)
            l1_fp8(3)
            l1_fp16(2)
            sigmoid(2)
            w8_dma(Bq, 7)
            l2(1)
            l1_fp8(4)
            l1_fp16(3)
            sigmoid(3)
            w16_dma(Bq, 5)
            l2(2)
            l1_fp8(5)
            l1_fp16(4)
            sigmoid(4)
            w16_dma(Bq, 7)
            l2(3)
            l1_fp8(6)
            l1_fp16(5)
            sigmoid(5)
            l2(4)
            l1_fp8(7)
            l1_fp16(6)
            sigmoid(6)
            l2(5)
            l1_fp16(7)
            sigmoid(7)
            l2(6)
            l2(7)

            # final b2-add + store, split in column halves on two engines so
            # the second half's DMA issue overlaps the first's transfer
            HB = BSH // 2
            outs = outp.tile([NT, BSH], dt.float32, tag="out", name="outs")
            nc.vector.tensor_scalar_add(
                outs[:, 0:HB], acc2[:NT, 0:HB], cns[0:NT, M1 : M1 + 1]
            )
            A.dma_start(out=outT[:, 0:HB], in_=outs[:, 0:HB])
            nc.vector.tensor_scalar_add(
                outs[:, HB:BSH], acc2[:NT, HB:BSH], cns[0:NT, M1 : M1 + 1]
            )
            nc.gpsimd.dma_start(out=outT[:, HB:BSH], in_=outs[:, HB:BSH])

    return nc


def _strip_start_barrier(nc):
    """Drop the all-engine drain + EVSEM barriers Tile emits in the 'main'
    block (~1.5-2us at start, ~1us at end). All Tile semaphores start at 0,
    and every cross-engine dependency inside the kernel is already
    semaphore-guarded. Optionally also drop the end-block cross-engine
    EVSEM barrier (each engine still drains its own queues + DMA lanes, so
    the output DMA is still awaited before NEFF completion)."""
    for fn in nc.m.functions:
        for bb in fn.blocks:
            if bb.name == "main":
                bb.instructions = [
                    i
                    for i in bb.instructions
                    if type(i).__name__ not in ("InstDrain", "InstEventSemaphore")
                ]
            elif STRIP_END and bb.name.endswith("_end"):
                bb.instructions = [
                    i
                    for i in bb.instructions
                    if type(i).__name__ != "InstEventSemaphore"
                ]


def _legalize_single_wait(nc):
    """This neuronxcc build allows at most ONE sync wait per instruction.
    Split extras onto same-engine no-ops placed immediately before."""
    import bass_rust

    for fn in nc.m.functions:
        for bb in fn.blocks:
            out, changed = [], False
            for ins in bb.instructions:
                si = ins.sync_info
                waits = list(si.on_wait) if si is not None else []
                if len(waits) > 1:
                    for i, w in enumerate(waits[:-1]):
                        out.append(
                            bass_rust.InstNoOp(
                                name=f"{ins.name}-sw{i}",
                                engine=ins.engine,
                                ins=[],
                                outs=[],
                                sync_info=bass_rust.SyncInfo(
                                    on_wait=[w], on_update=[]
                                ),
                            )
                        )
                    ins.sync_info = bass_rust.SyncInfo(
                        on_wait=[waits[-1]], on_update=list(si.on_update)
                    )
                    changed = True
                out.append(ins)
            if changed:
                bb.instructions = out


def _densify(w, rows_in, cols_out, n_in, n_out):
    dense = np.zeros((n_in, n_out), np.float32)
    np.add.at(dense, (np.asarray(rows_in), np.asarray(cols_out)), np.asarray(w))
    return dense


def _prep_inputs(x, w1, b1, w2, b2, conn1_out, conn1_in, conn2_out, conn2_in):
    import ml_dtypes

    f8 = ml_dtypes.float8_e4m3fn
    x = np.asarray(x, np.float32)
    W1 = _densify(w1, conn1_in, conn1_out, F, N1) * SW
    W2 = _densify(w2, conn2_in, conn2_out, N1, NT)

    r8 = FP8_PAIRS * 2 * P  # fp8 contraction rows
    # [j, s, p, m, q] -> [p, (m-ordered) m, j, s, q]
    w8v = W1[:r8].reshape(FP8_PAIRS, 2, P, M1, P).transpose(2, 3, 0, 1, 4)
    w8 = np.ascontiguousarray(w8v[:, MORDER]).reshape(P, -1).astype(f8)
    w16v = W1[r8:].reshape(FP16_PAIRS, 2, P, M1, P).transpose(2, 3, 0, 1, 4)
    w16 = np.ascontiguousarray(w16v[:, MORDER]).reshape(P, -1).astype(np.float16)
    w2c = np.ascontiguousarray(
        W2.reshape(M1, P, NT).transpose(1, 0, 2)
    ).reshape(P, M1 * NT).astype(np.float16)
    cn = np.zeros((P, M1 + 1), np.float32)
    cn[:, :M1] = np.asarray(b1, np.float32).reshape(M1, P).T
    cn[:NT, M1] = np.asarray(b2, np.float32)

    in_maps = []
    for c in range(NCORES):
        xs = x[c * BSH : (c + 1) * BSH].T * SX  # [F, BSH]
        x8v = np.ascontiguousarray(
            xs[:r8].reshape(FP8_PAIRS, 2, P, BSH).transpose(2, 0, 1, 3)
        ).reshape(P, -1).astype(f8)
        x16v = np.ascontiguousarray(
            xs[r8:].reshape(FP16_PAIRS, 2, P, BSH).transpose(2, 0, 1, 3)
        ).reshape(P, -1).astype(np.float16)
        in_maps.append(
            {"x8": x8v, "x16": x16v, "w8": w8, "w16": w16, "w2c": w2c, "cn": cn}
        )
    return in_maps


def _run(inputs, l1_bf16=True, trace=False, **run_kwargs):
    """Build (cached), run on the 8 NeuronCores, gather. Returns
    (out [4096, 8] float32, BassKernelResults)."""
    from concourse.bass_utils import run_bass_kernel_spmd

    if "nc" not in _CACHE:
        nc = _build()
        _strip_start_barrier(nc)
        _legalize_single_wait(nc)
        _CACHE["nc"] = nc
    nc = _CACHE["nc"]

    in_maps = _prep_inputs(**inputs)
    res = run_bass_kernel_spmd(
        nc, in_maps, list(range(NCORES)), trace=trace, **run_kwargs
    )
    out = np.empty((B, NT), np.float32)
    for c in range(NCORES):
        out[c * BSH : (c + 1) * BSH, :] = res.results[c]["outT"].T
    return out, res


def kernel(**inputs):
    out, _ = _run(inputs)
    return out


# revision 12
# speedup vs baseline: 1.0888x; 1.0888x over previous
"""Trainium2 Bass kernel for an edge-weighted two-layer sparse MLP (QBAF).

Math (identical to the gather/segment_sum reference):
    out = sigmoid(x @ W1 + b1) @ W2 + b2
with W1 [2048, 1024] / W2 [1024, 8] densified on host from the sparse
edge lists (scatter-add; duplicate edges accumulate like segment_sum).

Sharding: data-parallel over batch — 8 cores x 512 rows each; weights
replicated.

Precision/layout: the 2048-deep layer-1 contraction is split into 8
pairs of 256 rows. The first FP8_PAIRS pairs run as fp8(e4m3) DoubleRow
matmuls (256 contraction rows per 512-cycle instruction -- 2x the
fp16 PE rate); the rest run fp16. Both operand sets are pre-scaled
(x*16, W1*256) so everything shares one fp32 PSUM accumulation per
neuron tile; the sigmoid activation descales by 2^-12. Measured
end-to-end rel err ~1.6e-2 vs the fp32 reference (tolerance 2e-2).

Schedule: m-outer over the 8 neuron tiles, with each m's fp16 pairs
trailing one m behind its fp8 run (fp16 operands arrive later) and the
layer-2 matmul of m trailing two runs behind. PSUM banks therefore
complete staggered ~2.2us apart, and the 8 ACT-engine sigmoids (~0.7us
each) hide under remaining layer-1 work instead of serializing at the
tail. Inputs stream over both HWDGE rings (SP + ACT) in PE-consumption
order; the ACT ring interleaves sigmoids between its DMA issues. PE
warmup matmuls cover the pre-data window so the 1.2->2.4GHz clock ramp
(~3us of sustained PE activity) completes just as real data lands.
"""

import sys

import numpy as np

if "/opt/trn_rl_repo" not in sys.path:
    sys.path.insert(0, "/opt/trn_rl_repo")

B = 4096
F = 2048
N1 = 1024
NT = 8
NCORES = 8
BSH = B // NCORES  # 512 batch rows per core
P = 128
M1 = N1 // P  # 8 neuron tiles
PAIRS = F // (2 * P)  # 8 contraction pairs of 256 rows
FP8_PAIRS = 6
FP16_PAIRS = PAIRS - FP8_PAIRS
SX = 16.0  # x pre-scale (power of 2; keeps fp8 out of subnormals)
SW = 256.0  # W1 pre-scale
WARMUP = 48  # narrow PE matmuls: the 1.2->2.4GHz HAM clock ramp needs
# ~3.5-4.5us of CONTINUOUS PE activity, and a >2us stall after ramping
# DOWNCLOCKS again (measured), so warm up until the ramp is done AND all
# early-phase inputs have provably landed, then never stall >1us.
STRIP_END = False  # dropping the end-block EVSEM barrier deadlocks the
# end-block drains (they wait on events those EVSEMs set) -- keep it.

# DRAM m-block order: ring A streams m 0,2,4,6; ring B streams 1,3,5,7.
# Even-m blocks packed first so each ring's chunks are contiguous.
MORDER = [0, 2, 4, 6, 1, 3, 5, 7]
MPOS = {m: i for i, m in enumerate(MORDER)}

_CACHE = {}


def _build():
    import concourse.bass as bass
    import concourse.mybir as mybir
    import concourse.tile as tile

    dt = mybir.dt
    f8 = dt.float8e4
    f16 = dt.float16
    DR = mybir.MatmulPerfMode.DoubleRow
    SIG = mybir.ActivationFunctionType.Sigmoid

    nc = bass.Bass()
    x8 = nc.declare_dram_parameter("x8", [P, FP8_PAIRS * 2 * BSH], f8, isOutput=False)
    x16 = nc.declare_dram_parameter("x16", [P, FP16_PAIRS * 2 * BSH], f16, isOutput=False)
    w8 = nc.declare_dram_parameter("w8", [P, M1 * FP8_PAIRS * 2 * P], f8, isOutput=False)
    w16 = nc.declare_dram_parameter("w16", [P, M1 * FP16_PAIRS * 2 * P], f16, isOutput=False)
    w2c = nc.declare_dram_parameter("w2c", [P, M1 * NT], f16, isOutput=False)
    cn = nc.declare_dram_parameter("cn", [P, M1 + 1], dt.float32, isOutput=False)
    outT = nc.declare_dram_parameter("outT", [NT, BSH], dt.float32, isOutput=True)

    W8C = FP8_PAIRS * 2 * P  # w8 cols per m-block
    W16C = FP16_PAIRS * 2 * P

    with tile.TileContext(nc) as tc:
        with (
            tc.tile_pool(name="consts", bufs=1) as consts,
            tc.tile_pool(name="xp", bufs=1) as xp,
            tc.tile_pool(name="wp", bufs=1) as wp,
            tc.tile_pool(name="hp", bufs=M1) as hp,
            tc.tile_pool(name="outp", bufs=1) as outp,
            tc.tile_pool(name="ps", bufs=8, space="PSUM") as ps,
        ):
            x8t = {}  # pair j -> [P, 2, BSH] fp8 tile
            x16t = {}  # pair jj -> [P, 2, BSH] fp16 tile
            w8t = {}  # m -> list of (tile, tile_m_idx, j_lo, j_hi)
            w16t = {}  # m -> (tile, idx)

            def x8_dma(eng, j):
                t = xp.tile([P, 2, BSH], f8, tag=f"x8_{j}", name=f"x8_{j}")
                eng.dma_start(out=t[:], in_=x8[:, j * 2 * BSH : (j + 1) * 2 * BSH])
                x8t[j] = t

            def x16_dma(eng, jj):
                t = xp.tile([P, 2, BSH], f16, tag=f"x16_{jj}", name=f"x16_{jj}")
                eng.dma_start(out=t[:], in_=x16[:, jj * 2 * BSH : (jj + 1) * 2 * BSH])
                x16t[jj] = t

            def w8_dma(eng, m, jlo=0, jhi=FP8_PAIRS):
                np_ = jhi - jlo
                t = wp.tile([P, np_, 2, P], f8, tag=f"w8_{m}_{jlo}", name=f"w8_{m}_{jlo}")
                off = MPOS[m] * W8C + jlo * 2 * P
                eng.dma_start(out=t[:], in_=w8[:, off : off + np_ * 2 * P])
                w8t.setdefault(m, []).append((t, jlo, jhi))

            def w16_dma(eng, m):
                t = wp.tile([P, FP16_PAIRS, 2, P], f16, tag=f"w16_{m}", name=f"w16_{m}")
                off = MPOS[m] * W16C
                eng.dma_start(out=t[:], in_=w16[:, off : off + W16C])
                w16t[m] = t

            A = nc.sync
            Bq = nc.scalar

            # --- ring A (sync / SP): stream order = issue order
            w8_dma(A, 0, 0, 3)
            x8_dma(A, 1)
            x8_dma(A, 3)
            w8_dma(A, 0, 3, 6)
            x8_dma(A, 4)
            x8_dma(A, 5)
            w16_dma(A, 0)
            w8_dma(A, 2)
            w8_dma(A, 4)
            w8_dma(A, 6)
            w16_dma(A, 2)
            w16_dma(A, 4)
            w16_dma(A, 6)

            # --- ring B (scalar / ACT): DMA issues; the sigmoid chain and
            # the late weight blocks are interleaved further down.
            x8_dma(Bq, 0)
            x8_dma(Bq, 2)
            w8_dma(Bq, 1)
            x16_dma(Bq, 0)
            x16_dma(Bq, 1)
            cns = consts.tile([P, M1 + 1], dt.float32, tag="cn", name="cns")
            Bq.dma_start(out=cns[:], in_=cn[:])
            # dummy sigmoid: pulls the ACT table load off the critical path
            scr = consts.tile([P, 1], dt.float32, tag="scr", name="scr")
            bias0 = consts.tile([P, 1], dt.float32, tag="b0", name="bias0")
            nc.gpsimd.memset(bias0[:], 0.0)
            nc.scalar.activation(scr[:], bias0[:], SIG, bias=bias0[:], scale=1.0)
            w2s = consts.tile([P, M1 * NT], f16, tag="w2", name="w2s")
            Bq.dma_start(out=w2s[:], in_=w2c[:])
            w16_dma(Bq, 1)
            w8_dma(Bq, 3)

            hts = {}

            def sigmoid(m):
                ht = hp.tile([P, BSH], f16, tag="h", name=f"h{m}")
                nc.scalar.activation(
                    ht[:], accs[m][:], SIG, bias=cns[:, m : m + 1],
                    scale=1.0 / (SX * SW),
                )
                hts[m] = ht

            # --- PE program
            accs = [
                ps.tile([P, BSH], dt.float32, tag="acc", name=f"acc{m}")
                for m in range(M1)
            ]
            wsc = consts.tile([P, P], f16, tag="wsc", name="wsc")
            nc.gpsimd.memset(wsc[:], 0.0)
            for _ in range(WARMUP):
                nc.tensor.matmul(
                    accs[0][:, 0:P], wsc[:], wsc[:], start=True, stop=True
                )

            def l1_fp8(m):
                for t, jlo, jhi in w8t[m]:
                    for j in range(jlo, jhi):
                        nc.tensor.matmul(
                            accs[m][:],
                            t[:, j - jlo],
                            x8t[j][:],
                            start=(j == 0),
                            stop=False,
                            perf_mode=DR,
                        )

            def l1_fp16(m):
                t = w16t[m]
                for jj in range(FP16_PAIRS):
                    for s in range(2):
                        last = jj == FP16_PAIRS - 1 and s == 1
                        nc.tensor.matmul(
                            accs[m][:],
                            t[:, jj, s],
                            x16t[jj][:, s],
                            start=False,
                            stop=last,
                        )

            acc2 = ps.tile([P, BSH], dt.float32, tag="acc", name="acc2")

            def l2(m):
                nc.tensor.matmul(
                    acc2[:NT, :],
                    w2s[:, m * NT : (m + 1) * NT],
                    hts[m][:],
                    start=(m == 0),
                    stop=(m == M1 - 1),
                )

            # fp8 run of m; fp16 finish lags one m; L2 lags two. Sigmoids
            # (ACT) fire on each bank's stop; late B-ring DMA issues are
            # interleaved between them. Emission is chronological so Tile's
            # dependency tracking sees every accumulator write before its
            # sigmoid read.
            l1_fp8(0)
            l1_fp8(1)
            l1_fp16(0)  # stop m0
            sigmoid(0)
            w8_dma(Bq, 5)
            l1_fp8(2)
            l1_fp16(1)  # stop m1
            sigmoid(1)
            w16_dma(Bq, 3)
            l2(0)
            l1_fp8(3)
            l1_fp16(2)
            sigmoid(2)
            w8_dma(Bq, 7)
            l2(1)
            l1_fp8(4)
            l1_fp16(3)
            sigmoid(3)
            w16_dma(Bq, 5)
            l2(2)
            l1_fp8(5)
            l1_fp16(4)
            sigmoid(4)
            w16_dma(Bq, 7)
            l2(3)
            l1_fp8(6)
            l1_fp16(5)
            sigmoid(5)
            l2(4)
            l1_fp8(7)
            l1_fp16(6)
            sigmoid(6)
            l2(5)
            l1_fp16(7)
            sigmoid(7)
            l2(6)
            l2(7)

            # final b2-add + store: column halves on DVE and ACT in
            # parallel, each followed by its own HWDGE-ring store. Nothing
            # on gpsimd's software queue (its drain costs ~1.8us at exit).
            HB = BSH // 2
            outs = outp.tile([NT, BSH], dt.float32, tag="out", name="outs")
            nc.vector.tensor_scalar_add(
                outs[:, 0:HB], acc2[:NT, 0:HB], cns[0:NT, M1 : M1 + 1]
            )
            nc.scalar.activation(
                outs[:, HB:BSH],
                acc2[:NT, HB:BSH],
                mybir.ActivationFunctionType.Identity,
                bias=cns[0:NT, M1 : M1 + 1],
                scale=1.0,
            )
            A.dma_start(out=outT[:, 0:HB], in_=outs[:, 0:HB])
            Bq.dma_start(out=outT[:, HB:BSH], in_=outs[:, HB:BSH])

    return nc


def _strip_start_barrier(nc):
    """Drop the all-engine drain + EVSEM barriers Tile emits in the 'main'
    block (~1.5-2us at start, ~1us at end). All Tile semaphores start at 0,
    and every cross-engine dependency inside the kernel is already
    semaphore-guarded. Optionally also drop the end-block cross-engine
    EVSEM barrier (each engine still drains its own queues + DMA lanes, so
    the output DMA is still awaited before NEFF completion)."""
    for fn in nc.m.functions:
        for bb in fn.blocks:
            if bb.name == "main":
                bb.instructions = [
                    i
                    for i in bb.instructions
                    if type(i).__name__ not in ("InstDrain", "InstEventSemaphore")
                ]
            elif STRIP_END and bb.name.endswith("_end"):
                bb.instructions = [
                    i
                    for i in bb.instructions
                    if type(i).__name__ != "InstEventSemaphore"
                ]


def _legalize_single_wait(nc):
    """This neuronxcc build allows at most ONE sync wait per instruction.
    Split extras onto same-engine no-ops placed immediately before."""
    import bass_rust

    for fn in nc.m.functions:
        for bb in fn.blocks:
            out, changed = [], False
            for ins in bb.instructions:
                si = ins.sync_info
                waits = list(si.on_wait) if si is not None else []
                if len(waits) > 1:
                    for i, w in enumerate(waits[:-1]):
                        out.append(
                            bass_rust.InstNoOp(
                                name=f"{ins.name}-sw{i}",
                                engine=ins.engine,
                                ins=[],
                                outs=[],
                                sync_info=bass_rust.SyncInfo(
                                    on_wait=[w], on_update=[]
                                ),
                            )
                        )
                    ins.sync_info = bass_rust.SyncInfo(
                        on_wait=[waits[-1]], on_update=list(si.on_update)
                    )
                    changed = True
                out.append(ins)
            if changed:
                bb.instructions = out


def _densify(w, rows_in, cols_out, n_in, n_out):
    dense = np.zeros((n_in, n_out), np.float32)
    np.add.at(dense, (np.asarray(rows_in), np.asarray(cols_out)), np.asarray(w))
    return dense


def _prep_inputs(x, w1, b1, w2, b2, conn1_out, conn1_in, conn2_out, conn2_in):
    import ml_dtypes

    f8 = ml_dtypes.float8_e4m3fn
    x = np.asarray(x, np.float32)
    W1 = _densify(w1, conn1_in, conn1_out, F, N1) * SW
    W2 = _densify(w2, conn2_in, conn2_out, N1, NT)

    r8 = FP8_PAIRS * 2 * P  # fp8 contraction rows
    # [j, s, p, m, q] -> [p, (m-ordered) m, j, s, q]
    w8v = W1[:r8].reshape(FP8_PAIRS, 2, P, M1, P).transpose(2, 3, 0, 1, 4)
    w8 = np.ascontiguousarray(w8v[:, MORDER]).reshape(P, -1).astype(f8)
    w16v = W1[r8:].reshape(FP16_PAIRS, 2, P, M1, P).transpose(2, 3, 0, 1, 4)
    w16 = np.ascontiguousarray(w16v[:, MORDER]).reshape(P, -1).astype(np.float16)
    w2c = np.ascontiguousarray(
        W2.reshape(M1, P, NT).transpose(1, 0, 2)
    ).reshape(P, M1 * NT).astype(np.float16)
    cn = np.zeros((P, M1 + 1), np.float32)
    cn[:, :M1] = np.asarray(b1, np.float32).reshape(M1, P).T
    cn[:NT, M1] = np.asarray(b2, np.float32)

    in_maps = []
    for c in range(NCORES):
        xs = x[c * BSH : (c + 1) * BSH].T * SX  # [F, BSH]
        x8v = np.ascontiguousarray(
            xs[:r8].reshape(FP8_PAIRS, 2, P, BSH).transpose(2, 0, 1, 3)
        ).reshape(P, -1).astype(f8)
        x16v = np.ascontiguousarray(
            xs[r8:].reshape(FP16_PAIRS, 2, P, BSH).transpose(2, 0, 1, 3)
        ).reshape(P, -1).astype(np.float16)
        in_maps.append(
            {"x8": x8v, "x16": x16v, "w8": w8, "w16": w16, "w2c": w2c, "cn": cn}
        )
    return in_maps


def _run(inputs, l1_bf16=True, trace=False, **run_kwargs):
    """Build (cached), run on the 8 NeuronCores, gather. Returns
    (out [4096, 8] float32, BassKernelResults)."""
    from concourse.bass_utils import run_bass_kernel_spmd

    if "nc" not in _CACHE:
        nc = _build()
        _strip_start_barrier(nc)
        _legalize_single_wait(nc)
        _CACHE["nc"] = nc
    nc = _CACHE["nc"]

    in_maps = _prep_inputs(**inputs)
    res = run_bass_kernel_spmd(
        nc, in_maps, list(range(NCORES)), trace=trace, **run_kwargs
    )
    out = np.empty((B, NT), np.float32)
    for c in range(NCORES):
        out[c * BSH : (c + 1) * BSH, :] = res.results[c]["outT"].T
    return out, res


def kernel(**inputs):
    out, _ = _run(inputs)
    return out
